# revision 4
# baseline (speedup 1.0000x reference)
"""CRF negative-log-likelihood (mean) on 8 Trainium2 NeuronCores.

Strategy (data-parallel over batch, 64 sequences/core):

Denominator — forward algorithm in the multiplicative domain with a constant
per-step shift c (no per-step normalization; fp32 range is sufficient):
    P_0 = exp(em_0 - c) * exp(start + c)            [T=128, B_loc=64]
    P_i = (E^T P_{i-1}) o exp(em_i - c),  E = exp(transitions)
    den_b = (S-1)*c + ln( sum_t P_{S-1}[t,b] * exp(end[t]) )
Per step: one 128x128 @ 128x64 matmul (E stationary on the PE) and one DVE
tensor_tensor multiply out of PSUM with the precomputed exp(em - c) slice.
The sequence is walked from both ends (fwd with E, bwd with E^T) and joined
in the middle, so two independent serial chains keep both PE and DVE busy.
Emissions are host-permuted to [T, S, B_loc] so the chain needs no on-device
transposes; exp(em - c) is computed in bulk on the ACT engine off the
critical path.

Numerator — only its batch-sum is needed for the mean. All gather offsets
are precomputed on the HOST (pure functions of tags); the device does two
indirect-DMA element gathers (emissions at gold tags; transition/start/end
scores from a packed params table) plus reductions on the GPSIMD engine so
nothing sits in front of the chain on the PE/DVE queues.

Each core emits [colsum_b (64 values), numerator_sum]; the host combines:
    loss = sum_cores(sum_b ln colsum_b - num_sum) / B + (S-1)*c
"""

from contextlib import ExitStack

import numpy as np

import concourse.bass as bass
import concourse.bacc as bacc
import concourse.mybir as mybir
import concourse.tile as tile
from concourse.bass_utils import run_bass_kernel_spmd

F32 = mybir.dt.float32
BF16 = mybir.dt.bfloat16
I32 = mybir.dt.int32
AF = mybir.ActivationFunctionType
ALU = mybir.AluOpType
AX = mybir.AxisListType

B, S, T = 512, 512, 128
N_CORES = 8
BL = B // N_CORES
C_SHIFT = float(np.float32(np.log(128.0) + 0.5))
PW = 2 * T + 2  # packed params width: [trans | transT | start | end]


def _build_nc(chunk=32, w_dtype=BF16, state_dtype=BF16):
    assert S % chunk == 0
    MID = S // 2 - 1
    nc = bacc.Bacc("TRN2", target_bir_lowering=False, debug=False)

    emt = nc.declare_dram_parameter("emt", [T, S, BL], F32, isOutput=False)
    params_d = nc.declare_dram_parameter("params", [T, PW], F32, isOutput=False)
    offs_d = nc.declare_dram_parameter("offs", [BL, 2 * S + 1], I32, isOutput=False)
    out_d = nc.declare_dram_parameter("out", [BL + 1], F32, isOutput=True)

    with ExitStack() as ctx:
        tc = ctx.enter_context(tile.TileContext(nc))
        constp = ctx.enter_context(tc.tile_pool(name="const", bufs=1))
        emp = ctx.enter_context(tc.tile_pool(name="em", bufs=2))
        wp = ctx.enter_context(tc.tile_pool(name="w", bufs=1))
        statep = ctx.enter_context(tc.tile_pool(name="state", bufs=3))
        stateq = ctx.enter_context(tc.tile_pool(name="stateb", bufs=3))
        psump = ctx.enter_context(tc.tile_pool(name="psum", bufs=3, space="PSUM"))
        psumb = ctx.enter_context(tc.tile_pool(name="psumb", bufs=3, space="PSUM"))
        psumm = ctx.enter_context(tc.tile_pool(name="psumm", bufs=1, space="PSUM"))
        nump = ctx.enter_context(tc.tile_pool(name="num", bufs=1))

        # ---- constants ----
        posc_sb = constp.tile([T, 1], F32)
        nc.vector.memset(posc_sb[:], C_SHIFT)
        negc_sb = constp.tile([T, 1], F32)
        nc.vector.memset(negc_sb[:], -C_SHIFT)

        # warmup: first ACT-queue instruction triggers the Exp table load
        # immediately instead of gating the first real exp on it.
        warm_sb = constp.tile([1, 1], F32)
        nc.scalar.activation(warm_sb[:], posc_sb[0:1, 0:1], AF.Exp)

        # ---- W chunks (variable sizes: small boundary chunks first so the
        # chains start as early as possible, then alternate front/back) ----
        sizes = [4, 4, 12, 12, 16, 16]
        rem = S - sum(sizes)
        assert rem % chunk == 0
        sizes += [chunk] * (rem // chunk)
        front, back = 0, S
        spans = []  # (start, size)
        for j, sz in enumerate(sizes):
            if j % 2 == 0:
                spans.append((front, sz)); front += sz
            else:
                back -= sz; spans.append((back, sz))
        assert front == back
        w_tiles = [None] * len(spans)
        step_map = {}

        def emit_chunk(j):
            st, sz = spans[j]
            em_t = emp.tile([T, sz * BL], F32, tag=f"emchunk{min(j, 4)}")
            nc.sync.dma_start(
                em_t[:],
                emt[:, st:st + sz, :].rearrange("t s b -> t (s b)"),
            )
            w_t = wp.tile([T, sz * BL], w_dtype, tag=f"w{j}")
            nc.scalar.activation(w_t[:], em_t[:], AF.Exp, bias=negc_sb[:, 0:1])
            w_tiles[j] = w_t
            for q in range(sz):
                step_map[st + q] = (j, q)

        emit_chunk(0)

        # ---- packed params: one DMA for trans, transT, start, end ----
        params_sb = constp.tile([T, PW], F32)
        nc.sync.dma_start(params_sb[:], params_d[:])
        E_sb = constp.tile([T, T], state_dtype)
        nc.scalar.activation(E_sb[:], params_sb[:, 0:T], AF.Exp)
        startc_sb = constp.tile([T, 1], F32)
        nc.scalar.activation(startc_sb[:], params_sb[:, 2 * T:2 * T + 1], AF.Exp,
                             bias=posc_sb[:, 0:1])

        emit_chunk(1)

        ET_sb = constp.tile([T, T], state_dtype)
        nc.scalar.activation(ET_sb[:], params_sb[:, T:2 * T], AF.Exp)
        endexp_sb = constp.tile([T, 1], F32)
        nc.scalar.activation(endexp_sb[:], params_sb[:, 2 * T + 1:2 * T + 2], AF.Exp)

        # ---- numerator gather offsets (host-precomputed) ----
        offs_sb = nump.tile([BL, 2 * S + 1], I32)
        nc.sync.dma_start(offs_sb[:], offs_d[:])

        ones_sb = constp.tile([T, 1], F32)
        nc.vector.memset(ones_sb[:], 1.0)
        ones64 = constp.tile([BL, 1], F32)
        nc.vector.memset(ones64[:], 1.0)

        for _j in range(2, len(spans)):
            emit_chunk(_j)

        def w_slice(i):
            j, q = step_map[i]
            return w_tiles[j][:, q * BL:(q + 1) * BL]

        # ---- numerator gathers (gpsimd queue; DMA overlaps the chain) ----
        gv = nump.tile([BL, 2 * S + 1], F32)
        nc.gpsimd.indirect_dma_start(
            out=gv[:, 0:S], out_offset=None,
            in_=emt[:].rearrange("t s b -> (t s b)").rearrange("(x o) -> x o", o=1),
            in_offset=bass.IndirectOffsetOnAxis(ap=offs_sb[:, 0:S], axis=0),
        )
        nc.gpsimd.indirect_dma_start(
            out=gv[:, S:2 * S + 1], out_offset=None,
            in_=params_d[:].rearrange("u v -> (u v)").rearrange("(x o) -> x o", o=1),
            in_offset=bass.IndirectOffsetOnAxis(ap=offs_sb[:, S:2 * S + 1], axis=0),
        )

        # fused numerator reduce on the ACT engine (last in its queue, after
        # all chunk exps): accum_out sums gv along the free axis.
        gscratch = nump.tile([BL, 2 * S + 1], BF16)
        nsum = nump.tile([BL, 1], F32)
        nc.scalar.activation(gscratch[:], gv[:], AF.Identity, accum_out=nsum[:])

        # ---- chain states ----
        fstate = statep.tile([T, BL], state_dtype, tag="fstate")
        nc.vector.tensor_scalar(fstate[:], w_slice(0), startc_sb[:, 0:1], None,
                                ALU.mult)
        bstate = stateq.tile([T, BL], state_dtype, tag="bstate")
        nc.vector.tensor_scalar(bstate[:], w_slice(S - 1), endexp_sb[:, 0:1], None,
                                ALU.mult)

        fi = 1          # next fwd step: P_fi        (up to MID)
        bi = S - 2      # next bwd step: A_bi        (down to MID+1)
        while fi <= MID or bi >= MID + 1:
            if fi <= MID:
                q = psump.tile([T, BL], F32, tag="q")
                nc.tensor.matmul(q[:], lhsT=E_sb[:], rhs=fstate[:],
                                 start=True, stop=True)
                nf = statep.tile([T, BL], state_dtype, tag="fstate")
                nc.vector.tensor_tensor(nf[:], q[:], w_slice(fi), op=ALU.mult)
                fstate = nf
                fi += 1
            if bi >= MID + 1:
                qb = psumb.tile([T, BL], F32, tag="qb")
                nc.tensor.matmul(qb[:], lhsT=ET_sb[:], rhs=bstate[:],
                                 start=True, stop=True)
                nb = stateq.tile([T, BL], state_dtype, tag="bstate")
                nc.vector.tensor_tensor(nb[:], qb[:], w_slice(bi), op=ALU.mult)
                bstate = nb
                bi -= 1

        # join: Bt_MID = E @ A_{MID+1}; T_b = sum_t P_MID o Bt_MID
        qb = psumb.tile([T, BL], F32, tag="qb")
        nc.tensor.matmul(qb[:], lhsT=ET_sb[:], rhs=bstate[:], start=True, stop=True)
        pf = nump.tile([T, BL], F32)
        nc.vector.tensor_tensor(pf[:], qb[:], fstate[:], op=ALU.mult)
        colsum = psumm.tile([1, BL], F32, tag="colsum")
        nc.tensor.matmul(colsum[:], lhsT=ones_sb[:], rhs=pf[:], start=True, stop=True)

        numsum_ps = psumm.tile([1, 1], F32, tag="numsum")
        nc.tensor.matmul(numsum_ps[:], lhsT=ones64[:], rhs=nsum[:],
                         start=True, stop=True)

        out_sb = nump.tile([1, BL + 1], F32)
        nc.vector.tensor_copy(out_sb[:, 0:BL], colsum[:])
        nc.vector.tensor_copy(out_sb[:, BL:BL + 1], numsum_ps[:])
        nc.sync.dma_start(out_d[:].rearrange("(o x) -> o x", o=1), out_sb[:])

    return nc


_NC_CACHE = {}


def _get_nc():
    if "nc" not in _NC_CACHE:
        nc = _build_nc()
        nc.finalize()
        _NC_CACHE["nc"] = nc
    return _NC_CACHE["nc"]


def kernel(emissions, start_transitions, end_transitions, transitions, tags, mask,
           _trace=False):
    emissions = np.ascontiguousarray(np.asarray(emissions, dtype=np.float32))
    start_transitions = np.ascontiguousarray(
        np.asarray(start_transitions, dtype=np.float32))
    end_transitions = np.ascontiguousarray(
        np.asarray(end_transitions, dtype=np.float32))
    transitions = np.ascontiguousarray(np.asarray(transitions, dtype=np.float32))
    tags = np.ascontiguousarray(np.asarray(tags, dtype=np.int32))
    mask = np.asarray(mask)
    assert emissions.shape == (B, S, T) and tags.shape == (B, S)
    # setup_inputs() produces an all-ones mask; this kernel relies on it.
    assert np.all(mask == 1), "kernel assumes a full (all-ones) mask"

    params = np.empty((T, PW), dtype=np.float32)
    params[:, 0:T] = transitions
    params[:, T:2 * T] = transitions.T
    params[:, 2 * T] = start_transitions
    params[:, 2 * T + 1] = end_transitions

    in_maps = []
    for core in range(N_CORES):
        lo = core * BL
        tg = tags[lo:lo + BL].astype(np.int64)
        offs = np.empty((BL, 2 * S + 1), dtype=np.int32)
        # emissions gather: emt flat index (t, s, b_local)
        offs[:, 0:S] = (tg * (S * BL) + np.arange(S)[None, :] * BL
                        + np.arange(BL)[:, None])
        # transition scores from the packed params table (row stride PW)
        offs[:, S:2 * S - 1] = tg[:, :-1] * PW + tg[:, 1:]
        offs[:, 2 * S - 1] = tg[:, 0] * PW + 2 * T        # start_transitions
        offs[:, 2 * S] = tg[:, -1] * PW + 2 * T + 1       # end_transitions
        emt = np.ascontiguousarray(
            np.transpose(emissions[lo:lo + BL], (2, 1, 0)))  # [T, S, BL]
        in_maps.append({
            "emt": emt,
            "params": params,
            "offs": offs,
        })

    nc = _get_nc()
    res = run_bass_kernel_spmd(nc, in_maps, list(range(N_CORES)), trace=_trace)

    total = 0.0
    for r in res.results:
        o = np.asarray(r["out"], dtype=np.float64)
        total += float(np.sum(np.log(o[0:BL]))) - float(o[BL])
    loss = np.float32(total / B + (S - 1) * C_SHIFT)
    if _trace:
        return loss, res
    return loss


# revision 9
# speedup vs baseline: 1.0231x; 1.0231x over previous
"""CRF negative-log-likelihood (mean) on 8 Trainium2 NeuronCores.

Strategy (data-parallel over batch, 64 sequences/core):

Denominator — forward algorithm in the multiplicative domain with a constant
per-step shift c (no per-step normalization; fp32 range is sufficient):
    P_0 = exp(em_0 - c) * exp(start + c)            [T=128, B_loc=64]
    P_i = (E^T P_{i-1}) o exp(em_i - c),  E = exp(transitions)
    den_b = (S-1)*c + ln( sum_t P_{S-1}[t,b] * exp(end[t]) )
Per step: one 128x128 @ 128x64 matmul (E stationary on the PE) and one DVE
tensor_tensor multiply out of PSUM with the precomputed exp(em - c) slice.
The sequence is walked from both ends (fwd with E, bwd with E^T) and joined
in the middle, so two independent serial chains keep both PE and DVE busy.
Emissions are host-permuted to [T, S, B_loc] so the chain needs no on-device
transposes; exp(em - c) is computed in bulk on the ACT engine off the
critical path.

Numerator — only its batch-sum is needed for the mean. All gather offsets
are precomputed on the HOST (pure functions of tags); the device does two
indirect-DMA element gathers (emissions at gold tags; transition/start/end
scores from a packed params table) plus reductions on the GPSIMD engine so
nothing sits in front of the chain on the PE/DVE queues.

Each core emits [colsum_b (64 values), numerator_sum]; the host combines:
    loss = sum_cores(sum_b ln colsum_b - num_sum) / B + (S-1)*c
"""

from contextlib import ExitStack

import numpy as np

import concourse.bass as bass
import concourse.bacc as bacc
import concourse.mybir as mybir
import concourse.tile as tile
from concourse.bass_utils import run_bass_kernel_spmd

F32 = mybir.dt.float32
BF16 = mybir.dt.bfloat16
I32 = mybir.dt.int32
AF = mybir.ActivationFunctionType
ALU = mybir.AluOpType
AX = mybir.AxisListType

B, S, T = 512, 512, 128
N_CORES = 8
BL = B // N_CORES
C_SHIFT = float(np.float32(np.log(128.0) + 0.5))
PW = 2 * T + 2  # packed params width: [trans | transT | start | end]


def _build_nc(chunk=32, w_dtype=BF16, state_dtype=BF16):
    assert S % chunk == 0
    MID = S // 2 - 1
    nc = bacc.Bacc("TRN2", target_bir_lowering=False, debug=False)

    emt = nc.declare_dram_parameter("emt", [T, S, BL], F32, isOutput=False)
    params_d = nc.declare_dram_parameter("params", [T, PW], F32, isOutput=False)
    offs_d = nc.declare_dram_parameter("offs", [BL, 2 * S + 1], I32, isOutput=False)
    out_d = nc.declare_dram_parameter("out", [BL + 1], F32, isOutput=True)

    with ExitStack() as ctx:
        tc = ctx.enter_context(tile.TileContext(nc))
        constp = ctx.enter_context(tc.tile_pool(name="const", bufs=1))
        emp = ctx.enter_context(tc.tile_pool(name="em", bufs=2))
        wp = ctx.enter_context(tc.tile_pool(name="w", bufs=1))
        statep = ctx.enter_context(tc.tile_pool(name="state", bufs=3))
        stateq = ctx.enter_context(tc.tile_pool(name="stateb", bufs=3))
        psump = ctx.enter_context(tc.tile_pool(name="psum", bufs=3, space="PSUM"))
        psumb = ctx.enter_context(tc.tile_pool(name="psumb", bufs=3, space="PSUM"))
        psumm = ctx.enter_context(tc.tile_pool(name="psumm", bufs=1, space="PSUM"))
        nump = ctx.enter_context(tc.tile_pool(name="num", bufs=1))

        # ---- constants ----
        posc_sb = constp.tile([T, 1], F32)
        nc.vector.memset(posc_sb[:], C_SHIFT)
        negc_sb = constp.tile([T, 1], F32)
        nc.vector.memset(negc_sb[:], -C_SHIFT)

        # warmup: first ACT-queue instruction triggers the Exp table load
        # immediately instead of gating the first real exp on it.
        warm_sb = constp.tile([1, 1], F32)
        nc.scalar.activation(warm_sb[:], posc_sb[0:1, 0:1], AF.Exp)

        # ---- W chunks (variable sizes: small boundary chunks first so the
        # chains start as early as possible, then alternate front/back) ----
        sizes = [8, 8, 16, 16, 24, 24]
        rem = S - sum(sizes)
        assert rem % chunk == 0
        sizes += [chunk] * (rem // chunk)
        front, back = 0, S
        spans = []  # (start, size)
        for j, sz in enumerate(sizes):
            if j % 2 == 0:
                spans.append((front, sz)); front += sz
            else:
                back -= sz; spans.append((back, sz))
        assert front == back
        w_tiles = [None] * len(spans)
        step_map = {}

        def emit_chunk(j):
            st, sz = spans[j]
            em_t = emp.tile([T, sz * BL], F32, tag=f"emchunk{min(j, 4)}")
            nc.sync.dma_start(
                em_t[:],
                emt[:, st:st + sz, :].rearrange("t s b -> t (s b)"),
            )
            w_t = wp.tile([T, sz * BL], w_dtype, tag=f"w{j}")
            nc.scalar.activation(w_t[:], em_t[:], AF.Exp, bias=negc_sb[:, 0:1])
            w_tiles[j] = w_t
            for q in range(sz):
                step_map[st + q] = (j, q)

        emit_chunk(0)

        # ---- packed params: one DMA for trans, transT, start, end ----
        params_sb = constp.tile([T, PW], F32)
        nc.sync.dma_start(params_sb[:], params_d[:])
        E_sb = constp.tile([T, T], state_dtype)
        nc.scalar.activation(E_sb[:], params_sb[:, 0:T], AF.Exp)
        startc_sb = constp.tile([T, 1], F32)
        nc.scalar.activation(startc_sb[:], params_sb[:, 2 * T:2 * T + 1], AF.Exp,
                             bias=posc_sb[:, 0:1])
        ET_sb = constp.tile([T, T], state_dtype)
        nc.scalar.activation(ET_sb[:], params_sb[:, T:2 * T], AF.Exp)
        endexp_sb = constp.tile([T, 1], F32)
        nc.scalar.activation(endexp_sb[:], params_sb[:, 2 * T + 1:2 * T + 2], AF.Exp)

        emit_chunk(1)

        # ---- numerator gather offsets (host-precomputed) ----
        offs_sb = nump.tile([BL, 2 * S + 1], I32)
        nc.sync.dma_start(offs_sb[:], offs_d[:])

        ones_sb = constp.tile([T, 1], F32)
        nc.vector.memset(ones_sb[:], 1.0)

        for _j in range(2, len(spans)):
            emit_chunk(_j)

        def w_slice(i):
            j, q = step_map[i]
            return w_tiles[j][:, q * BL:(q + 1) * BL]

        # ---- numerator gathers (gpsimd queue; DMA overlaps the chain) ----
        gv = nump.tile([BL, 2 * S + 1], F32)
        nc.gpsimd.indirect_dma_start(
            out=gv[:, 0:S], out_offset=None,
            in_=emt[:].rearrange("t s b -> (t s b)").rearrange("(x o) -> x o", o=1),
            in_offset=bass.IndirectOffsetOnAxis(ap=offs_sb[:, 0:S], axis=0),
        )
        nc.gpsimd.indirect_dma_start(
            out=gv[:, S:2 * S + 1], out_offset=None,
            in_=params_d[:].rearrange("u v -> (u v)").rearrange("(x o) -> x o", o=1),
            in_offset=bass.IndirectOffsetOnAxis(ap=offs_sb[:, S:2 * S + 1], axis=0),
        )

        # fused numerator reduce on the ACT engine (accum_out sums gv along
        # the free axis), then a partition reduce + copy on GPSIMD — the
        # whole numerator stays off the PE/DVE queues so the scheduler
        # cannot stall the chain on it.
        gscratch = nump.tile([BL, 2 * S + 1], BF16)
        nsum = nump.tile([BL, 1], F32)
        nc.scalar.activation(gscratch[:], gv[:], AF.Identity, accum_out=nsum[:])
        out_sb = nump.tile([1, BL + 1], F32)
        nc.gpsimd.tensor_reduce(out_sb[:, BL:BL + 1], nsum[:], axis=AX.C,
                                op=ALU.add)

        # ---- chain states ----
        fstate = statep.tile([T, BL], state_dtype, tag="fstate")
        nc.vector.tensor_scalar(fstate[:], w_slice(0), startc_sb[:, 0:1], None,
                                ALU.mult)
        bstate = stateq.tile([T, BL], state_dtype, tag="bstate")
        nc.vector.tensor_scalar(bstate[:], w_slice(S - 1), endexp_sb[:, 0:1], None,
                                ALU.mult)

        fi = 1          # next fwd step: P_fi        (up to MID)
        bi = S - 2      # next bwd step: A_bi        (down to MID+1)
        while fi <= MID or bi >= MID + 1:
            if fi <= MID:
                q = psump.tile([T, BL], F32, tag="q")
                nc.tensor.matmul(q[:], lhsT=E_sb[:], rhs=fstate[:],
                                 start=True, stop=True)
                nf = statep.tile([T, BL], state_dtype, tag="fstate")
                nc.vector.tensor_tensor(nf[:], q[:], w_slice(fi), op=ALU.mult)
                fstate = nf
                fi += 1
            if bi >= MID + 1:
                qb = psumb.tile([T, BL], F32, tag="qb")
                nc.tensor.matmul(qb[:], lhsT=ET_sb[:], rhs=bstate[:],
                                 start=True, stop=True)
                nb = stateq.tile([T, BL], state_dtype, tag="bstate")
                nc.vector.tensor_tensor(nb[:], qb[:], w_slice(bi), op=ALU.mult)
                bstate = nb
                bi -= 1

        # join: Bt_MID = E @ A_{MID+1}; T_b = sum_t P_MID o Bt_MID
        qb = psumb.tile([T, BL], F32, tag="qb")
        nc.tensor.matmul(qb[:], lhsT=ET_sb[:], rhs=bstate[:], start=True, stop=True)
        pf = nump.tile([T, BL], F32)
        nc.vector.tensor_tensor(pf[:], qb[:], fstate[:], op=ALU.mult)
        colsum = psumm.tile([1, BL], F32, tag="colsum")
        nc.tensor.matmul(colsum[:], lhsT=ones_sb[:], rhs=pf[:], start=True, stop=True)

        nc.vector.tensor_copy(out_sb[:, 0:BL], colsum[:])
        nc.sync.dma_start(out_d[:].rearrange("(o x) -> o x", o=1), out_sb[:])

    return nc


_NC_CACHE = {}


def _get_nc():
    if "nc" not in _NC_CACHE:
        nc = _build_nc()
        nc.finalize()
        _NC_CACHE["nc"] = nc
    return _NC_CACHE["nc"]


def kernel(emissions, start_transitions, end_transitions, transitions, tags, mask,
           _trace=False):
    emissions = np.ascontiguousarray(np.asarray(emissions, dtype=np.float32))
    start_transitions = np.ascontiguousarray(
        np.asarray(start_transitions, dtype=np.float32))
    end_transitions = np.ascontiguousarray(
        np.asarray(end_transitions, dtype=np.float32))
    transitions = np.ascontiguousarray(np.asarray(transitions, dtype=np.float32))
    tags = np.ascontiguousarray(np.asarray(tags, dtype=np.int32))
    mask = np.asarray(mask)
    assert emissions.shape == (B, S, T) and tags.shape == (B, S)
    # setup_inputs() produces an all-ones mask; this kernel relies on it.
    assert np.all(mask == 1), "kernel assumes a full (all-ones) mask"

    params = np.empty((T, PW), dtype=np.float32)
    params[:, 0:T] = transitions
    params[:, T:2 * T] = transitions.T
    params[:, 2 * T] = start_transitions
    params[:, 2 * T + 1] = end_transitions

    in_maps = []
    for core in range(N_CORES):
        lo = core * BL
        tg = tags[lo:lo + BL].astype(np.int64)
        offs = np.empty((BL, 2 * S + 1), dtype=np.int32)
        # emissions gather: emt flat index (t, s, b_local)
        offs[:, 0:S] = (tg * (S * BL) + np.arange(S)[None, :] * BL
                        + np.arange(BL)[:, None])
        # transition scores from the packed params table (row stride PW)
        offs[:, S:2 * S - 1] = tg[:, :-1] * PW + tg[:, 1:]
        offs[:, 2 * S - 1] = tg[:, 0] * PW + 2 * T        # start_transitions
        offs[:, 2 * S] = tg[:, -1] * PW + 2 * T + 1       # end_transitions
        emt = np.ascontiguousarray(
            np.transpose(emissions[lo:lo + BL], (2, 1, 0)))  # [T, S, BL]
        in_maps.append({
            "emt": emt,
            "params": params,
            "offs": offs,
        })

    nc = _get_nc()
    res = run_bass_kernel_spmd(nc, in_maps, list(range(N_CORES)), trace=_trace)

    total = 0.0
    for r in res.results:
        o = np.asarray(r["out"], dtype=np.float64)
        total += float(np.sum(np.log(o[0:BL]))) - float(o[BL])
    loss = np.float32(total / B + (S - 1) * C_SHIFT)
    if _trace:
        return loss, res
    return loss


# revision 23
# speedup vs baseline: 1.0231x; 1.0001x over previous
"""CRF negative-log-likelihood (mean) on 8 Trainium2 NeuronCores.

Strategy (data-parallel over batch, 64 sequences/core):

Denominator — forward algorithm in the multiplicative domain with a constant
per-step shift c (no per-step normalization; fp32 range is sufficient):
    P_0 = exp(em_0 - c) * exp(start + c)            [T=128, B_loc=64]
    P_i = (E^T P_{i-1}) o exp(em_i - c),  E = exp(transitions)
    den_b = (S-1)*c + ln( sum_t P_{S-1}[t,b] * exp(end[t]) )
Per step: one 128x128 @ 128x64 matmul (E stationary on the PE) and one DVE
tensor_tensor multiply out of PSUM with the precomputed exp(em - c) slice.
The sequence is walked from both ends (fwd with E, bwd with E^T) and joined
in the middle, so two independent serial chains keep both PE and DVE busy.
Emissions are host-permuted to [T, S, B_loc] so the chain needs no on-device
transposes; exp(em - c) is computed in bulk on the ACT engine off the
critical path.

Numerator — only its batch-sum is needed for the mean. All gather offsets
are precomputed on the HOST (pure functions of tags); the device does two
indirect-DMA element gathers (emissions at gold tags; transition/start/end
scores from a packed params table) plus reductions on the GPSIMD engine so
nothing sits in front of the chain on the PE/DVE queues.

Each core emits [colsum_b (64 values), numerator_sum]; the host combines:
    loss = sum_cores(sum_b ln colsum_b - num_sum) / B + (S-1)*c
"""

from contextlib import ExitStack

import numpy as np

import concourse.bass as bass
import concourse.bacc as bacc
import concourse.mybir as mybir
import concourse.tile as tile
from concourse.bass_utils import run_bass_kernel_spmd

F32 = mybir.dt.float32
BF16 = mybir.dt.bfloat16
I32 = mybir.dt.int32
AF = mybir.ActivationFunctionType
ALU = mybir.AluOpType
AX = mybir.AxisListType

B, S, T = 512, 512, 128
N_CORES = 8
BL = B // N_CORES
C_SHIFT = float(np.float32(np.log(128.0) + 0.5))
PW = 2 * T + 3  # packed params width: [trans | transT | start | end | end-c]


def _build_nc(chunk=32, w_dtype=BF16, state_dtype=BF16):
    assert S % chunk == 0
    MID = S // 2 - 1
    nc = bacc.Bacc("TRN2", target_bir_lowering=False, debug=False)

    emt = nc.declare_dram_parameter("emt", [T, S, BL], F32, isOutput=False)
    params_d = nc.declare_dram_parameter("params", [T, PW], F32, isOutput=False)
    offs_d = nc.declare_dram_parameter("offs", [BL, 2 * S + 1], I32, isOutput=False)
    out_d = nc.declare_dram_parameter("out", [BL], F32, isOutput=True)
    out2_d = nc.declare_dram_parameter("out2", [1], F32, isOutput=True)

    with ExitStack() as ctx:
        tc = ctx.enter_context(tile.TileContext(nc))
        constp = ctx.enter_context(tc.tile_pool(name="const", bufs=1))
        emp = ctx.enter_context(tc.tile_pool(name="em", bufs=2))
        wp = ctx.enter_context(tc.tile_pool(name="w", bufs=1))
        statep = ctx.enter_context(tc.tile_pool(name="state", bufs=3))
        stateq = ctx.enter_context(tc.tile_pool(name="stateb", bufs=3))
        psump = ctx.enter_context(tc.tile_pool(name="psum", bufs=3, space="PSUM"))
        psumb = ctx.enter_context(tc.tile_pool(name="psumb", bufs=3, space="PSUM"))
        psumm = ctx.enter_context(tc.tile_pool(name="psumm", bufs=1, space="PSUM"))
        nump = ctx.enter_context(tc.tile_pool(name="num", bufs=1))

        # ---- constants ----
        posc_sb = constp.tile([T, 1], F32)
        nc.vector.memset(posc_sb[:], C_SHIFT)
        negc_sb = constp.tile([T, 1], F32)
        nc.vector.memset(negc_sb[:], -C_SHIFT)

        # warmup: first ACT-queue instruction triggers the Exp table load
        # immediately instead of gating the first real exp on it.
        warm_sb = constp.tile([1, 1], F32)
        nc.scalar.activation(warm_sb[:], posc_sb[0:1, 0:1], AF.Exp)

        # ---- W chunks (variable sizes: small boundary chunks first so the
        # chains start as early as possible, then alternate front/back) ----
        sizes = [8, 8, 16, 16, 24, 24]
        rem = S - sum(sizes)
        assert rem % chunk == 0
        sizes += [chunk] * (rem // chunk)
        front, back = 0, S
        spans = []  # (start, size)
        for j, sz in enumerate(sizes):
            if j % 2 == 0:
                spans.append((front, sz)); front += sz
            else:
                back -= sz; spans.append((back, sz))
        assert front == back
        w_tiles = [None] * len(spans)
        em_tiles = [None] * len(spans)
        step_map = {}

        def emit_chunk(j):
            st, sz = spans[j]
            em_t = emp.tile([T, sz * BL], F32, tag=f"emchunk{min(j, 4)}")
            em_tiles[j] = em_t
            nc.sync.dma_start(
                em_t[:],
                emt[:, st:st + sz, :].rearrange("t s b -> t (s b)"),
            )
            w_t = wp.tile([T, sz * BL], w_dtype, tag=f"w{j}")
            nc.scalar.activation(w_t[:], em_t[:], AF.Exp, bias=negc_sb[:, 0:1])
            w_tiles[j] = w_t
            for q in range(sz):
                step_map[st + q] = (j, q)

        emit_chunk(0)

        # ---- packed params: one DMA for trans, transT, start, end ----
        params_sb = constp.tile([T, PW], F32)
        nc.sync.dma_start(params_sb[:], params_d[:])
        E_sb = constp.tile([T, T], state_dtype)
        nc.scalar.activation(E_sb[:], params_sb[:, 0:T], AF.Exp)
        # fstate = exp(em_0 + start): the +c of the start shift cancels the
        # -c of the emission shift, so raw em chunk0 with bias=start works.
        fstate = statep.tile([T, BL], state_dtype, tag="fstate")
        nc.scalar.activation(fstate[:], em_tiles[0][:, 0:BL], AF.Exp,
                             bias=params_sb[:, 2 * T:2 * T + 1])
        ET_sb = constp.tile([T, T], state_dtype)
        nc.scalar.activation(ET_sb[:], params_sb[:, T:2 * T], AF.Exp)

        emit_chunk(1)
        # bstate = exp(em_{S-1} - c + end); host packs params col 2T+1 as
        # (end - c) so it can be used as the bias directly.
        bstate = stateq.tile([T, BL], state_dtype, tag="bstate")
        nc.scalar.activation(bstate[:], em_tiles[1][:, (sizes[1] - 1) * BL:],
                             AF.Exp, bias=params_sb[:, 2 * T + 2:2 * T + 3])

        # ---- numerator gather offsets (host-precomputed) ----
        offs_sb = nump.tile([BL, 2 * S + 1], I32)
        nc.sync.dma_start(offs_sb[:], offs_d[:])

        ones_sb = constp.tile([T, 1], BF16)
        nc.vector.memset(ones_sb[:], 1.0)

        for _j in range(2, len(spans)):
            emit_chunk(_j)

        def w_slice(i):
            j, q = step_map[i]
            return w_tiles[j][:, q * BL:(q + 1) * BL]

        # ---- numerator gathers (gpsimd queue; DMA overlaps the chain) ----
        gv = nump.tile([BL, 2 * S + 1], F32)
        nc.gpsimd.indirect_dma_start(
            out=gv[:, 0:S], out_offset=None,
            in_=emt[:].rearrange("t s b -> (t s b)").rearrange("(x o) -> x o", o=1),
            in_offset=bass.IndirectOffsetOnAxis(ap=offs_sb[:, 0:S], axis=0),
        )
        nc.gpsimd.indirect_dma_start(
            out=gv[:, S:2 * S + 1], out_offset=None,
            in_=params_d[:].rearrange("u v -> (u v)").rearrange("(x o) -> x o", o=1),
            in_offset=bass.IndirectOffsetOnAxis(ap=offs_sb[:, S:2 * S + 1], axis=0),
        )

        # numerator reduce as a GPSIMD-only add tree: stays entirely off the
        # PE/DVE/ACT queues (no chain stalls, no shared-accumulator hazards)
        # and runs mid-kernel, fully hidden behind the chain.
        cur = nump.tile([BL, S], F32)
        nc.gpsimd.tensor_tensor(cur[:], gv[:, 0:S], gv[:, S:2 * S], op=ALU.add)
        w = S
        while w > 1:
            half = w // 2
            nxt = nump.tile([BL, half], F32, tag=f"tree{half}")
            nc.gpsimd.tensor_tensor(nxt[:], cur[:, 0:half], cur[:, half:w],
                                    op=ALU.add)
            cur = nxt
            w = half
        nsum = nump.tile([BL, 1], F32)
        nc.gpsimd.tensor_tensor(nsum[:], cur[:], gv[:, 2 * S:2 * S + 1],
                                op=ALU.add)
        num_sb = nump.tile([1, 1], F32)
        nc.gpsimd.tensor_reduce(num_sb[:], nsum[:], axis=AX.C, op=ALU.add)
        # numerator result DMAs out mid-kernel, hidden behind the chain
        nc.sync.dma_start(out2_d[:].rearrange("(o x) -> o x", o=1), num_sb[:])

        fi = 1          # next fwd step: P_fi        (up to MID)
        bi = S - 2      # next bwd step: A_bi        (down to MID+1)
        while fi <= MID or bi >= MID + 1:
            if fi <= MID:
                q = psump.tile([T, BL], F32, tag="q")
                nc.tensor.matmul(q[:], lhsT=E_sb[:], rhs=fstate[:],
                                 start=True, stop=True)
                nf = statep.tile([T, BL], state_dtype, tag="fstate")
                nc.vector.tensor_tensor(nf[:], q[:], w_slice(fi), op=ALU.mult)
                fstate = nf
                fi += 1
            if bi >= MID + 1:
                qb = psumb.tile([T, BL], F32, tag="qb")
                nc.tensor.matmul(qb[:], lhsT=ET_sb[:], rhs=bstate[:],
                                 start=True, stop=True)
                nb = stateq.tile([T, BL], state_dtype, tag="bstate")
                nc.vector.tensor_tensor(nb[:], qb[:], w_slice(bi), op=ALU.mult)
                bstate = nb
                bi -= 1

        # join: Bt_MID = E @ A_{MID+1}; T_b = sum_t P_MID o Bt_MID
        qb = psumb.tile([T, BL], F32, tag="qb")
        nc.tensor.matmul(qb[:], lhsT=ET_sb[:], rhs=bstate[:], start=True, stop=True)
        pf = nump.tile([T, BL], BF16)
        nc.vector.tensor_tensor(pf[:], qb[:], fstate[:], op=ALU.mult)
        colsum = psumm.tile([1, BL], F32, tag="colsum")
        nc.tensor.matmul(colsum[:], lhsT=ones_sb[:], rhs=pf[:], start=True, stop=True)

        out_sb = nump.tile([1, BL], F32)
        nc.vector.tensor_copy(out_sb[:], colsum[:])
        nc.sync.dma_start(out_d[:].rearrange("(o x) -> o x", o=1), out_sb[:])

    return nc


_NC_CACHE = {}


def _get_nc():
    if "nc" not in _NC_CACHE:
        nc = _build_nc()
        nc.finalize()
        _NC_CACHE["nc"] = nc
    return _NC_CACHE["nc"]


def kernel(emissions, start_transitions, end_transitions, transitions, tags, mask,
           _trace=False):
    emissions = np.ascontiguousarray(np.asarray(emissions, dtype=np.float32))
    start_transitions = np.ascontiguousarray(
        np.asarray(start_transitions, dtype=np.float32))
    end_transitions = np.ascontiguousarray(
        np.asarray(end_transitions, dtype=np.float32))
    transitions = np.ascontiguousarray(np.asarray(transitions, dtype=np.float32))
    tags = np.ascontiguousarray(np.asarray(tags, dtype=np.int32))
    mask = np.asarray(mask)
    assert emissions.shape == (B, S, T) and tags.shape == (B, S)
    # setup_inputs() produces an all-ones mask; this kernel relies on it.
    assert np.all(mask == 1), "kernel assumes a full (all-ones) mask"

    params = np.empty((T, PW), dtype=np.float32)
    params[:, 0:T] = transitions
    params[:, T:2 * T] = transitions.T
    params[:, 2 * T] = start_transitions
    params[:, 2 * T + 1] = end_transitions
    params[:, 2 * T + 2] = end_transitions - np.float32(C_SHIFT)

    in_maps = []
    for core in range(N_CORES):
        lo = core * BL
        tg = tags[lo:lo + BL].astype(np.int64)
        offs = np.empty((BL, 2 * S + 1), dtype=np.int32)
        # emissions gather: emt flat index (t, s, b_local)
        offs[:, 0:S] = (tg * (S * BL) + np.arange(S)[None, :] * BL
                        + np.arange(BL)[:, None])
        # transition scores from the packed params table (row stride PW)
        offs[:, S:2 * S - 1] = tg[:, :-1] * PW + tg[:, 1:]
        offs[:, 2 * S - 1] = tg[:, 0] * PW + 2 * T        # start_transitions
        offs[:, 2 * S] = tg[:, -1] * PW + 2 * T + 1       # end_transitions
        emt = np.ascontiguousarray(
            np.transpose(emissions[lo:lo + BL], (2, 1, 0)))  # [T, S, BL]
        in_maps.append({
            "emt": emt,
            "params": params,
            "offs": offs,
        })

    nc = _get_nc()
    res = run_bass_kernel_spmd(nc, in_maps, list(range(N_CORES)), trace=_trace)

    total = 0.0
    for r in res.results:
        o = np.asarray(r["out"], dtype=np.float64)
        total += float(np.sum(np.log(o))) - float(np.asarray(r["out2"])[0])
    loss = np.float32(total / B + (S - 1) * C_SHIFT)
    if _trace:
        return loss, res
    return loss
